# revision 1
# baseline (speedup 1.0000x reference)
"""Trainium2 Bass kernel for an AttentionBlock (B=16, C=256, N=1024 tokens,
4 heads x d_k=64), data-parallel over batch across 8 NeuronCores.

Layout strategy: all device math runs in "transposed" token-last layout.
x[b] arrives as [C, N] which is exactly xf^T, the natural stationary operand
(lhsT) for every matmul, and the output y^T [C, N] is exactly the layout the
problem wants back ([B, C, H, W]).  No transposes anywhere.

Per batch element, per core:
  qk^T [512, N] = W_qk^T @ xf^T     (q pre-scaled by dk^-0.5, +b_q on copy;
                                     b_k dropped: constant-over-keys terms
                                     cancel in softmax)
  v    [N, 4, 128]                  (tokens on partitions; cols 64..127 = 1.0
                                     so the PV matmul emits the softmax
                                     denominator replicated on rows 64..127)
  S^T  [j, i] per head = k^T.T @ q^T  -- two d_k=64 heads packed in the
                                     128-row PE array via row strips
  P^T = exp(S^T)                    (no max subtraction: |scores| <= ~11,
                                     exp <= ~1.4e4, safe in fp32)
  O^T[128, i] = v_aug.T @ P^T       accumulated over 8 j-tiles; rows 0-63 are
                                     unnormalized O^T, rows 64-127 all carry
                                     the denominator -> 64-lane reciprocal +
                                     one multiply normalizes, no PE involved
  y^T = W_out^T @ res^T + (b_out + b_v @ W_out) + x^T

All matmuls run as float32r (full fp32 storage, 1 cycle/row on the PE vs 4
for plain fp32); producers feeding matmuls tag outputs f32r via bitcast to
satisfy the BIR verifier.

Scheduling: engines execute their queues in order, so emission order is the
performance lever.  The attention stream is globally software-pipelined with
a one-stage skew (each stage emits S^T + exp for stage k, then the PV of
stage k-1) so the PE never head-of-line blocks on an exp; group completions
(normalize, out-projection) are emitted when the group's last PV retires,
and ALL qk/v generation work beyond a minimal prologue is dispatched as one
small chunk per attention stage (_gen_sched) so no lump ever bubbles the
ACT engine.  ACT (exp, ~68us busy) is the bottleneck engine; everything
else hides behind it.
"""

import numpy as np

N_CORES = 8
B, C = 16, 256
N = 1024  # H*W = 32*32
NH, DK = 4, 64
BPC = B // N_CORES  # batch elements per core
P = 128
KT = 2  # C / 128 contraction tiles
ISZ = 512  # i-tile (query) width
NI = N // ISZ  # 2
NJ = N // P  # 8 key tiles
SCALE = DK ** -0.5

_CACHE = {}


def _build_module():
    import concourse.bass as bass  # noqa: F401
    import concourse.mybir as mybir
    import concourse.tile as tile
    from concourse import bacc

    f32 = mybir.dt.float32
    f32r = mybir.dt.float32r
    ADD = mybir.AluOpType.add
    EXP = mybir.ActivationFunctionType.Exp

    nc = bacc.Bacc(
        "TRN2",
        debug=False,
        enable_asserts=False,
        target_bir_lowering=False,
        num_devices=N_CORES,
    )

    x_d = nc.dram_tensor("x", [BPC, C, N], f32, kind="ExternalInput").ap()
    # all weights packed: [p, kt, 0:512]=wqk, [512:768]=wv, [768:1024]=wout
    wall_d = nc.dram_tensor("wall", [P, KT, 4 * P + NH * DK + C], f32,
                            kind="ExternalInput").ap()
    # small constants packed: [p, 0:2]=bq, [2:4]=beff, [4:68]=ones
    sm_d = nc.dram_tensor("sm", [P, 4 + DK], f32, kind="ExternalInput").ap()
    y_d = nc.dram_tensor("y", [BPC, C, N], f32, kind="ExternalOutput").ap()

    with tile.TileContext(nc) as tc:
        with (
            tc.tile_pool(name="const", bufs=1) as const,
            tc.tile_pool(name="xp", bufs=2) as xp,
            tc.tile_pool(name="qkp", bufs=2) as qkp,
            tc.tile_pool(name="vp", bufs=1) as vp,
            tc.tile_pool(name="ptp", bufs=8) as ptp,
            tc.tile_pool(name="resp", bufs=2) as resp,
            tc.tile_pool(name="smp", bufs=4) as smp,
            tc.tile_pool(name="outp", bufs=4) as outp,
            tc.tile_pool(name="psg", bufs=2, space="PSUM") as psg,
            tc.tile_pool(name="pss", bufs=2, space="PSUM") as pss,
            tc.tile_pool(name="psv", bufs=2, space="PSUM") as psv,
        ):
            # --- constants: one packed weight DMA on the SP queue, one
            # packed small-constant DMA on the idle gpsimd queue ---
            sm_sb = const.tile([P, 4 + DK], f32, tag="sm")
            nc.gpsimd.dma_start(sm_sb.bitcast(f32r), sm_d.bitcast(f32r))
            bq_sb = sm_sb[:, 0:2]
            beff_sb = sm_sb[:, 2:4]
            ones_sb = sm_sb[:, 4:]
            wall_sb = const.tile([P, KT, 4 * P + NH * DK + C], f32, tag="wall")
            # wqk first (it gates the first matmuls), then wv+wout
            nc.sync.dma_start(
                wall_sb[:, :, 0 : 4 * P].bitcast(f32r),
                wall_d[:, :, 0 : 4 * P].bitcast(f32r),
            )
            nc.sync.dma_start(
                wall_sb[:, :, 4 * P :].bitcast(f32r),
                wall_d[:, :, 4 * P :].bitcast(f32r),
            )
            wqk_sb = wall_sb[:, :, 0 : 4 * P]
            wv_sb = wall_sb[:, :, 4 * P : 4 * P + NH * DK]
            wout_sb = wall_sb[:, :, 4 * P + NH * DK :]
            # persistent v tiles (one per batch element); ones block
            # [*, jt, h, 64:128] written once via a broadcast DVE copy
            v_tiles = [
                vp.tile([P, NJ, NH, 2 * DK], f32, tag=f"v{vb}", name=f"v{vb}")
                for vb in range(BPC)
            ]

            def emit_xload(b):
                x_sb = xp.tile([P, KT, N], f32, tag="x", name=f"x{b}")
                # split in halves so the first qk/v matmuls start sooner
                for half in range(2):
                    sl = slice(half * (N // 2), (half + 1) * (N // 2))
                    nc.scalar.dma_start(
                        x_sb[:, :, sl].bitcast(f32r),
                        x_d[b]
                        .rearrange("(kt p) n -> p kt n", p=P)[:, :, sl]
                        .bitcast(f32r),
                    )
                return x_sb

            def alloc_qk(b):
                return [
                    qkp.tile([P, N], f32, tag=f"qk{t}", name=f"qk{t}_{b}")
                    for t in range(4)
                ]

            def emit_qkgen(b, x_sb, qk_sb=None, i_list=None, t_list=None):
                # feature tiles [q01, k01, q23, k23]; i outer so the first
                # half of x unblocks all four tiles
                if qk_sb is None:
                    qk_sb = alloc_qk(b)
                for i in (range(NI) if i_list is None else i_list):
                    for t in (range(4) if t_list is None else t_list):
                        qt = qk_sb[t]
                        ps = psg.tile([P, ISZ], f32, tag="gen", name="gqk")
                        for kt in range(KT):
                            nc.tensor.matmul(
                                ps,
                                lhsT=wqk_sb[:, kt, t * P : (t + 1) * P].bitcast(f32r),
                                rhs=x_sb[:, kt, i * ISZ : (i + 1) * ISZ].bitcast(f32r),
                                start=(kt == 0),
                                stop=(kt == KT - 1),
                            )
                        dst = qt[:, i * ISZ : (i + 1) * ISZ]
                        if t % 2 == 0:  # q tile: add pre-scaled bias
                            hp = t // 2
                            nc.vector.tensor_scalar_add(
                                dst.bitcast(f32r), ps, bq_sb[:, hp : hp + 1]
                            )
                        elif b == 0 and i == 0 and t == 1:
                            # first k tile: copy j0's 128 columns first so
                            # the first S^T (and hence the first exp) can
                            # issue ~0.4us earlier
                            nc.vector.tensor_copy(
                                dst[:, 0:P].bitcast(f32r), ps[:, 0:P]
                            )
                            nc.vector.tensor_copy(
                                dst[:, P:].bitcast(f32r), ps[:, P:]
                            )
                        else:
                            nc.vector.tensor_copy(dst.bitcast(f32r), ps)
                return qk_sb

            def emit_vgen(b, x_sb, jt_list=None, ones_range=None):
                v_sb = v_tiles[b]
                if ones_range is not None:
                    s, c = ones_range
                    nc.vector.tensor_copy(
                        v_sb[:, s : s + c, :, DK:].bitcast(f32r),
                        ones_sb.rearrange(
                            "p (a b d) -> p a b d", a=1, b=1
                        ).to_broadcast([P, c, NH, DK]),
                    )
                for jt in (range(NJ) if jt_list is None else jt_list):
                    ps = psg.tile([P, ISZ], f32, tag="gen", name="gv")
                    psv_view = ps[:, : NH * DK]
                    for kt in range(KT):
                        nc.tensor.matmul(
                            psv_view,
                            lhsT=x_sb[:, kt, jt * P : (jt + 1) * P].bitcast(f32r),
                            rhs=wv_sb[:, kt, :].bitcast(f32r),
                            start=(kt == 0),
                            stop=(kt == KT - 1),
                        )
                    nc.vector.tensor_copy(
                        v_sb[:, jt, :, 0:DK].bitcast(f32r),
                        psv_view.rearrange("p (h d) -> p h d", h=NH),
                    )
                return v_sb

            def emit_pv_stage(stg):
                b2, i2, hp2, jt, pvs, v_sb, pt, res_sb = stg
                for h in range(2):
                    nc.tensor.matmul(
                        pvs[h],
                        lhsT=v_sb[:, jt, 2 * hp2 + h, :].bitcast(f32r),
                        rhs=pt[:, h, :].bitcast(f32r),
                        start=(jt == 0),
                        stop=(jt == NJ - 1),
                    )
                if jt == NJ - 1:
                    on_group_complete(b2, i2, hp2, pvs, res_sb)

            def on_group_complete(b, i, hp, pvs, res_sb):
                # normalize: denominator replicated on rows 64..127
                for h in range(2):
                    rcp = smp.tile([DK, ISZ], f32, tag=f"rcp{h}", name=f"rcp{h}")
                    nc.vector.reciprocal(rcp, pvs[h][DK : 2 * DK, :])
                    nc.vector.tensor_mul(
                        res_sb[
                            h * DK : (h + 1) * DK, hp, i * ISZ : (i + 1) * ISZ
                        ].bitcast(f32r),
                        pvs[h][0:DK, :],
                        rcp,
                    )
                if hp == 1 and (b, i) not in _deferred_op:
                    tiles = [None, None]
                    for kt in range(KT):
                        emit_outproj_kt(i, res_sb, kt, tiles)
                    emit_outproj_tail(b, i, x_sbs[b], tiles)

            def emit_outproj_mt(b, i, mt):
                # full out-projection for one output row-tile: 2 accumulate
                # matmuls + bias/residual + store.  Used by the stage-hook
                # chunks that spread an i-boundary's out-projection.
                res_sb = res_sbs[b]
                x_sb = x_sbs[b]
                ps = psg.tile([P, ISZ], f32, tag="gen", name="gop")
                for kt in range(KT):
                    nc.tensor.matmul(
                        ps,
                        lhsT=wout_sb[:, kt, mt * P : (mt + 1) * P].bitcast(f32r),
                        rhs=res_sb[:, kt, i * ISZ : (i + 1) * ISZ].bitcast(f32r),
                        start=(kt == 0),
                        stop=(kt == KT - 1),
                    )
                y_sb = outp.tile([P, ISZ], f32, tag="y", name="y")
                nc.vector.scalar_tensor_tensor(
                    out=y_sb,
                    in0=ps,
                    scalar=beff_sb[:, mt : mt + 1],
                    in1=x_sb[:, mt, i * ISZ : (i + 1) * ISZ],
                    op0=ADD,
                    op1=ADD,
                )
                nc.sync.dma_start(
                    y_d[b, mt * P : (mt + 1) * P, i * ISZ : (i + 1) * ISZ],
                    y_sb,
                )

            def emit_outproj_kt(i, res_sb, kt, tiles):
                # one contraction step for both output row-tiles; kt=0 only
                # needs hp=0's normalized rows, so it can run while hp=1's
                # attention stream is still in flight
                for mt in range(KT):
                    if kt == 0:
                        tiles[mt] = psg.tile([P, ISZ], f32, tag="gen", name="gout")
                    nc.tensor.matmul(
                        tiles[mt],
                        lhsT=wout_sb[:, kt, mt * P : (mt + 1) * P].bitcast(f32r),
                        rhs=res_sb[:, kt, i * ISZ : (i + 1) * ISZ].bitcast(f32r),
                        start=(kt == 0),
                        stop=(kt == KT - 1),
                    )

            def emit_outproj_tail(b, i, x_sb, tiles):
                for mt in range(KT):
                    y_sb = outp.tile([P, ISZ], f32, tag="y", name="y")
                    nc.vector.scalar_tensor_tensor(
                        out=y_sb,
                        in0=tiles[mt],
                        scalar=beff_sb[:, mt : mt + 1],
                        in1=x_sb[:, mt, i * ISZ : (i + 1) * ISZ],
                        op0=ADD,
                        op1=ADD,
                    )
                    nc.sync.dma_start(
                        y_d[b, mt * P : (mt + 1) * P, i * ISZ : (i + 1) * ISZ],
                        y_sb,
                    )

            # globally software-pipelined attention: each stage emits
            # S^T + exp for (g, jt) and only then the PV of the previous
            # stage, so the in-order PE queue never stalls on an exp.
            # Group completions (normalize, out-proj, next batch's qk/v
            # generation) ride along when a stage's last PV is emitted.
            x_sbs = {0: emit_xload(0)}
            # minimal prologue: just q01/k01 on the first half of x and
            # the first v tiles, so exp starts as early as possible; all
            # remaining generation work is dispatched one chunk per
            # attention stage via _gen_sched below.
            qk_sbs = {0: emit_qkgen(0, x_sbs[0], i_list=[0], t_list=[0, 1])}
            v_sbs = {0: emit_vgen(0, x_sbs[0], jt_list=[0, 1, 2],
                                  ones_range=(0, 4))}
            res_sbs = {}
            pending = None
            # stage-hook schedule: (b, i, hp, jt) -> (target_batch, kind,
            # kwargs).  Batch 0's leftover generation spreads through the
            # pipeline-fill stages; batch b+1's full generation spreads one
            # chunk per steady-state stage of batch b's i=1 groups (the PE
            # has headroom there once warm; a lump would bubble ACT).
            _gen_sched = {
                (0, 0, 0, 0): (0, "v", dict(jt_list=[3])),
                (0, 0, 0, 1): (0, "qk", dict(i_list=[1], t_list=[1])),
                (0, 0, 0, 2): (0, "qk", dict(i_list=[0], t_list=[2])),
                (0, 0, 0, 3): (0, "qk", dict(i_list=[0], t_list=[3])),
                (0, 0, 0, 4): (0, "v", dict(jt_list=[4],
                                            ones_range=(4, 4))),
                (0, 0, 0, 5): (0, "v", dict(jt_list=[5])),
                (0, 0, 0, 6): (0, "v", dict(jt_list=[6])),
                (0, 0, 0, 7): (0, "v", dict(jt_list=[7])),
                (0, 0, 1, 1): (0, "qk", dict(i_list=[1], t_list=[3])),
                (0, 0, 1, 2): (0, "qk", dict(i_list=[1], t_list=[0])),
                (0, 0, 1, 3): (0, "qk", dict(i_list=[1], t_list=[2])),
            }
            # every i-boundary out-projection except the kernel-final one
            # is spread as two per-mt chunks into the stages right after its
            # group completes (the completion traces earlier in the same
            # stage, so the data dependency is already satisfied).  To free
            # those slots, four of batch b+1's qk chunks move into the empty
            # fill-phase slots of batch 0's second group.
            _deferred_op = set()
            for nb in range(1, BPC):
                pb = nb - 1
                _deferred_op.add((pb, 0))
                _gen_sched[(pb, 1, 0, 0)] = (pb, "op", dict(i=0, mt=0))
                _gen_sched[(pb, 1, 0, 1)] = (pb, "op", dict(i=0, mt=1))
                _deferred_op.add((pb, NI - 1))
                _gen_sched[(nb, 0, 0, 0)] = (pb, "op", dict(i=NI - 1, mt=0))
                _gen_sched[(nb, 0, 0, 1)] = (pb, "op", dict(i=NI - 1, mt=1))
                if nb + 1 < BPC:
                    pass  # deeper pipelines would continue the pattern
            _deferred_op.add((BPC - 1, 0))
            _gen_sched[(BPC - 1, 1, 0, 0)] = (BPC - 1, "op", dict(i=0, mt=0))
            _gen_sched[(BPC - 1, 1, 0, 1)] = (BPC - 1, "op", dict(i=0, mt=1))
            for nb in range(1, BPC):
                pb = nb - 1
                # batch nb's qk chunks for i=0 ride the free fill-phase
                # slots (traced after x(nb) is loaded at batch pb's start)
                for tt in range(4):
                    _gen_sched[(0, 0, 1, 4 + tt)] = (
                        nb, "qk", dict(i_list=[0], t_list=[tt])
                    )
                chunks = [(nb, "v", dict(jt_list=[0], ones_range=(0, NJ)))]
                for tt in range(4):
                    chunks.append((nb, "qk", dict(i_list=[1], t_list=[tt])))
                for jj in range(1, NJ):
                    chunks.append((nb, "v", dict(jt_list=[jj])))
                # alternate qk and v chunks across the remaining free slots
                # of batch pb's i=1 groups to even the per-stage PE load
                order = []
                qks = [c for c in chunks if c[1] == "qk"]
                vs = [c for c in chunks if c[1] != "qk"]
                while qks or vs:
                    if qks:
                        order.append(qks.pop(0))
                    if vs:
                        order.append(vs.pop(0))
                slots = [
                    (pb, NI - 1, hp2, jj)
                    for hp2 in range(2)
                    for jj in range(NJ)
                    if (pb, NI - 1, hp2, jj) not in _gen_sched
                ]
                for slot, c in zip(slots, order):
                    _gen_sched[slot] = c
            for b in range(BPC):
                res_sbs[b] = resp.tile([P, KT, N], f32, tag="res", name=f"res{b}")
                for i in range(NI):
                    if i == 0 and b + 1 < BPC:
                        x_sbs[b + 1] = emit_xload(b + 1)
                    for hp in range(2):
                        q_t = qk_sbs[b][2 * hp]
                        k_t = qk_sbs[b][2 * hp + 1]
                        pvs = [
                            psv.tile([P, ISZ], f32, tag="pv", name=f"pv{h}")
                            for h in range(2)
                        ]
                        for jt in range(NJ):
                            st = pss.tile([P, 2, ISZ], f32, tag="st", name="st")
                            for h in range(2):
                                nc.tensor.matmul(
                                    st[:, h, :],
                                    lhsT=k_t[
                                        h * DK : (h + 1) * DK,
                                        jt * P : (jt + 1) * P,
                                    ].bitcast(f32r),
                                    rhs=q_t[
                                        h * DK : (h + 1) * DK,
                                        i * ISZ : (i + 1) * ISZ,
                                    ].bitcast(f32r),
                                )
                            pt = ptp.tile([P, 2, ISZ], f32, tag="pt", name="pt")
                            nc.scalar.activation(pt.bitcast(f32r), st, EXP)
                            if pending is not None:
                                emit_pv_stage(pending)
                            pending = (
                                b, i, hp, jt, pvs, v_sbs[b], pt, res_sbs[b]
                            )
                            chunk = _gen_sched.pop((b, i, hp, jt), None)
                            if chunk is not None:
                                nb, kind, args = chunk
                                if kind == "op":
                                    emit_outproj_mt(nb, **args)
                                elif kind == "qk":
                                    if nb not in qk_sbs:
                                        qk_sbs[nb] = alloc_qk(nb)
                                    emit_qkgen(nb, x_sbs[nb],
                                               qk_sb=qk_sbs[nb], **args)
                                else:
                                    v_sbs[nb] = emit_vgen(
                                        nb, x_sbs[nb], **args
                                    )
            emit_pv_stage(pending)

    nc.compile()
    return nc


def _prep_weights(W_qkv, b_qkv, W_out, b_out):
    """Host-side weight reshuffles (cheap, [256, 768]-sized)."""
    Wr = np.ascontiguousarray(W_qkv, dtype=np.float32).reshape(C, NH, 3, DK)
    br = np.ascontiguousarray(b_qkv, dtype=np.float32).reshape(NH, 3, DK)
    # feature tiles: [q0|q1], [k0|k1], [q2|q3], [k2|k3]; q pre-scaled
    cols = []
    for hp in range(2):
        cols.append(Wr[:, 2 * hp, 0] * SCALE)
        cols.append(Wr[:, 2 * hp + 1, 0] * SCALE)
        cols.append(Wr[:, 2 * hp, 1])
        cols.append(Wr[:, 2 * hp + 1, 1])
    wqk = np.concatenate(cols, axis=1)  # [C, 512]
    bq = np.stack(
        [
            np.concatenate([br[2 * hp, 0], br[2 * hp + 1, 0]]) * SCALE
            for hp in range(2)
        ],
        axis=1,
    )  # [128, 2]
    wv = np.concatenate([Wr[:, h, 2] for h in range(NH)], axis=1)  # [C, 256]
    bv = np.concatenate([br[h, 2] for h in range(NH)])  # [256]
    W_out = np.ascontiguousarray(W_out, dtype=np.float32)
    b_eff = (b_out + bv @ W_out).astype(np.float32)  # [256]
    beff = b_eff.reshape(KT, P).T.copy()  # [128, 2] col=mt
    return (
        np.ascontiguousarray(wqk, dtype=np.float32),
        np.ascontiguousarray(bq, dtype=np.float32),
        np.ascontiguousarray(wv, dtype=np.float32),
        W_out,
        np.ascontiguousarray(beff, dtype=np.float32),
    )


def _device_inputs(x3, W_qkv, b_qkv, W_out, b_out):
    wqk, bq, wv, wout, beff = _prep_weights(
        np.asarray(W_qkv), np.asarray(b_qkv), np.asarray(W_out), np.asarray(b_out)
    )
    def to_pkm(w):  # [C, M] -> [P, KT, M] with row kt*P+p on (p, kt)
        return w.reshape(KT, P, -1).transpose(1, 0, 2)

    wall = np.ascontiguousarray(
        np.concatenate([to_pkm(wqk), to_pkm(wv), to_pkm(wout)], axis=2),
        dtype=np.float32,
    )
    sm = np.ascontiguousarray(
        np.concatenate([bq, beff, np.ones((P, DK), np.float32)], axis=1),
        dtype=np.float32,
    )
    in_maps = []
    for c in range(N_CORES):
        in_maps.append(
            {
                "x": np.ascontiguousarray(x3[c * BPC : (c + 1) * BPC]),
                "wall": wall,
                "sm": sm,
            }
        )
    return in_maps


def kernel(x, W_qkv, b_qkv, W_out, b_out):
    from concourse.bass_utils import run_bass_kernel_spmd

    if "nc" not in _CACHE:
        _CACHE["nc"] = _build_module()
    nc = _CACHE["nc"]

    x = np.ascontiguousarray(np.asarray(x), dtype=np.float32)
    Bx, Cx, Hx, Wx = x.shape
    x3 = x.reshape(Bx, Cx, Hx * Wx)
    in_maps = _device_inputs(x3, W_qkv, b_qkv, W_out, b_out)

    res = run_bass_kernel_spmd(nc, in_maps, core_ids=list(range(N_CORES)))
    y = np.concatenate([r["y"] for r in res.results], axis=0)  # [16, 256, 1024]
    return y.reshape(Bx, Cx, Hx, Wx).astype(np.float32)



# revision 3
# speedup vs baseline: 1.4822x; 1.4822x over previous
"""Trainium2 Bass kernel for an AttentionBlock (B=16, C=256, N=1024 tokens,
4 heads x d_k=64), data-parallel over batch across 8 NeuronCores.

Layout strategy: all device math runs in "transposed" token-last layout.
x[b] arrives as [C, N] which is exactly xf^T, the natural stationary operand
(lhsT) for every matmul, and the output y^T [C, N] is exactly the layout the
problem wants back ([B, C, H, W]).  No transposes anywhere.

Per batch element, per core:
  qk^T [512, N] = W_qk^T @ xf^T     (q pre-scaled by dk^-0.5, +b_q on copy;
                                     b_k dropped: constant-over-keys terms
                                     cancel in softmax)
  v    [N, 4, 128]                  (tokens on partitions; cols 64..127 = 1.0
                                     so the PV matmul emits the softmax
                                     denominator replicated on rows 64..127)
  S^T  [j, i] per head = k^T.T @ q^T  -- two d_k=64 heads packed in the
                                     128-row PE array via row strips
  P^T = exp(S^T)                    (no max subtraction: |scores| <= ~11,
                                     exp <= ~1.4e4, safe in fp32)
  O^T[128, i] = v_aug.T @ P^T       accumulated over 8 j-tiles; rows 0-63 are
                                     unnormalized O^T, rows 64-127 all carry
                                     the denominator -> 64-lane reciprocal +
                                     one multiply normalizes, no PE involved
  y^T = W_out^T @ res^T + (b_out + b_v @ W_out) + x^T

All matmuls run as float32r (full fp32 storage, 1 cycle/row on the PE vs 4
for plain fp32); producers feeding matmuls tag outputs f32r via bitcast to
satisfy the BIR verifier.

Scheduling: engines execute their queues in order, so emission order is the
performance lever.  The attention stream is globally software-pipelined with
a one-stage skew (each stage emits S^T + exp for stage k, then the PV of
stage k-1) so the PE never head-of-line blocks on an exp; group completions
(normalize, out-projection) are emitted when the group's last PV retires,
and ALL qk/v generation work beyond a minimal prologue is dispatched as one
small chunk per attention stage (_gen_sched) so no lump ever bubbles the
ACT engine.  ACT (exp, ~68us busy) is the bottleneck engine; everything
else hides behind it.
"""

import numpy as np

N_CORES = 8
B, C = 16, 256
N = 1024  # H*W = 32*32
NH, DK = 4, 64
BPC = B // N_CORES  # batch elements per core
P = 128
KT = 2  # C / 128 contraction tiles
ISZ = 512  # i-tile (query) width
NI = N // ISZ  # 2
NJ = N // P  # 8 key tiles
SCALE = DK ** -0.5

_CACHE = {}

# Schraudolph exp approximation on DVE: float bits = int32(s*A + B)
SCH_A32 = 12102203.161561485  # 2^23 / ln 2
SCH_B32 = 1065112200.0  # tuned against np.exp on the score range


def _build_module(dve_exp=(), kv_act=False):
    import concourse.bass as bass  # noqa: F401
    import concourse.mybir as mybir
    import concourse.tile as tile
    from concourse import bacc

    f32 = mybir.dt.float32
    f32r = mybir.dt.float32r
    i32 = mybir.dt.int32
    ADD = mybir.AluOpType.add
    MULT = mybir.AluOpType.mult
    DIV = mybir.AluOpType.divide
    EXP = mybir.ActivationFunctionType.Exp
    CPY = mybir.ActivationFunctionType.Copy
    dve_exp = set(dve_exp)

    nc = bacc.Bacc(
        "TRN2",
        debug=False,
        enable_asserts=False,
        target_bir_lowering=False,
        num_devices=N_CORES,
    )

    x_d = nc.dram_tensor("x", [BPC, C, N], f32, kind="ExternalInput").ap()
    # all weights packed: [p, kt, 0:512]=wqk, [512:768]=wv, [768:1024]=wout
    wall_d = nc.dram_tensor("wall", [P, KT, 4 * P + NH * DK + C], f32,
                            kind="ExternalInput").ap()
    # small constants packed: [p, 0:2]=bq, [2:4]=beff, [4:68]=ones
    sm_d = nc.dram_tensor("sm", [P, 4 + DK], f32, kind="ExternalInput").ap()
    y_d = nc.dram_tensor("y", [BPC, C, N], f32, kind="ExternalOutput").ap()

    with tile.TileContext(nc) as tc:
        with (
            tc.tile_pool(name="const", bufs=1) as const,
            tc.tile_pool(name="xp", bufs=2) as xp,
            tc.tile_pool(name="qkp", bufs=2) as qkp,
            tc.tile_pool(name="vp", bufs=1) as vp,
            tc.tile_pool(name="ptp", bufs=8) as ptp,
            tc.tile_pool(name="resp", bufs=2) as resp,
            tc.tile_pool(name="smp", bufs=4) as smp,
            tc.tile_pool(name="outp", bufs=4) as outp,
            tc.tile_pool(name="psg", bufs=2, space="PSUM") as psg,
            tc.tile_pool(name="pss", bufs=2, space="PSUM") as pss,
            tc.tile_pool(name="psv", bufs=2, space="PSUM") as psv,
        ):
            # --- constants: one packed weight DMA on the SP queue, one
            # packed small-constant DMA on the idle gpsimd queue ---
            sm_sb = const.tile([P, 4 + DK], f32, tag="sm")
            nc.gpsimd.dma_start(sm_sb.bitcast(f32r), sm_d.bitcast(f32r))
            bq_sb = sm_sb[:, 0:2]
            beff_sb = sm_sb[:, 2:4]
            ones_sb = sm_sb[:, 4:]
            wall_sb = const.tile([P, KT, 4 * P + NH * DK + C], f32, tag="wall")
            # q01/k01 columns first (they gate the first matmuls and the
            # first exp), then the rest of wqk, then wv+wout
            nc.sync.dma_start(
                wall_sb[:, :, 0 : 2 * P].bitcast(f32r),
                wall_d[:, :, 0 : 2 * P].bitcast(f32r),
            )
            nc.sync.dma_start(
                wall_sb[:, :, 2 * P : 4 * P].bitcast(f32r),
                wall_d[:, :, 2 * P : 4 * P].bitcast(f32r),
            )
            nc.sync.dma_start(
                wall_sb[:, :, 4 * P :].bitcast(f32r),
                wall_d[:, :, 4 * P :].bitcast(f32r),
            )
            wqk_sb = wall_sb[:, :, 0 : 4 * P]
            wv_sb = wall_sb[:, :, 4 * P : 4 * P + NH * DK]
            wout_sb = wall_sb[:, :, 4 * P + NH * DK :]
            # persistent v tiles (one per batch element); ones block
            # [*, jt, h, 64:128] written once via a broadcast DVE copy
            v_tiles = [
                vp.tile([P, NJ, NH, 2 * DK], f32, tag=f"v{vb}", name=f"v{vb}")
                for vb in range(BPC)
            ]

            def emit_xload(b):
                x_sb = xp.tile([P, KT, N], f32, tag="x", name=f"x{b}")
                # split in halves so the first qk/v matmuls start sooner
                for half in range(2):
                    sl = slice(half * (N // 2), (half + 1) * (N // 2))
                    nc.scalar.dma_start(
                        x_sb[:, :, sl].bitcast(f32r),
                        x_d[b]
                        .rearrange("(kt p) n -> p kt n", p=P)[:, :, sl]
                        .bitcast(f32r),
                    )
                return x_sb

            def alloc_qk(b):
                return [
                    qkp.tile([P, N], f32, tag=f"qk{t}", name=f"qk{t}_{b}")
                    for t in range(4)
                ]

            def emit_qkgen(b, x_sb, qk_sb=None, i_list=None, t_list=None):
                # feature tiles [q01, k01, q23, k23]; i outer so the first
                # half of x unblocks all four tiles
                if qk_sb is None:
                    qk_sb = alloc_qk(b)
                for i in (range(NI) if i_list is None else i_list):
                    for t in (range(4) if t_list is None else t_list):
                        qt = qk_sb[t]
                        ps = psg.tile([P, ISZ], f32, tag="gen", name="gqk")
                        for kt in range(KT):
                            nc.tensor.matmul(
                                ps,
                                lhsT=wqk_sb[:, kt, t * P : (t + 1) * P].bitcast(f32r),
                                rhs=x_sb[:, kt, i * ISZ : (i + 1) * ISZ].bitcast(f32r),
                                start=(kt == 0),
                                stop=(kt == KT - 1),
                            )
                        dst = qt[:, i * ISZ : (i + 1) * ISZ]
                        if t % 2 == 0:  # q tile: add pre-scaled bias
                            hp = t // 2
                            nc.vector.tensor_scalar_add(
                                dst.bitcast(f32r), ps, bq_sb[:, hp : hp + 1]
                            )
                        elif b == 0 and i == 0 and t == 1:
                            # first k tile: copy j0's 128 columns first so
                            # the first S^T (and hence the first exp) can
                            # issue ~0.4us earlier
                            nc.vector.tensor_copy(
                                dst[:, 0:P].bitcast(f32r), ps[:, 0:P]
                            )
                            nc.vector.tensor_copy(
                                dst[:, P:].bitcast(f32r), ps[:, P:]
                            )
                        elif kv_act:
                            nc.scalar.activation(dst.bitcast(f32r), ps, CPY)
                        else:
                            nc.vector.tensor_copy(dst.bitcast(f32r), ps)
                return qk_sb

            def emit_vgen(b, x_sb, jt_list=None, ones_range=None):
                v_sb = v_tiles[b]
                if ones_range is not None:
                    s, c = ones_range
                    nc.vector.tensor_copy(
                        v_sb[:, s : s + c, :, DK:].bitcast(f32r),
                        ones_sb.rearrange(
                            "p (a b d) -> p a b d", a=1, b=1
                        ).to_broadcast([P, c, NH, DK]),
                    )
                for jt in (range(NJ) if jt_list is None else jt_list):
                    ps = psg.tile([P, ISZ], f32, tag="gen", name="gv")
                    psv_view = ps[:, : NH * DK]
                    for kt in range(KT):
                        nc.tensor.matmul(
                            psv_view,
                            lhsT=x_sb[:, kt, jt * P : (jt + 1) * P].bitcast(f32r),
                            rhs=wv_sb[:, kt, :].bitcast(f32r),
                            start=(kt == 0),
                            stop=(kt == KT - 1),
                        )
                    if kv_act:
                        nc.scalar.activation(
                            v_sb[:, jt, :, 0:DK].bitcast(f32r),
                            psv_view.rearrange("p (h d) -> p h d", h=NH),
                            CPY,
                        )
                    else:
                        nc.vector.tensor_copy(
                            v_sb[:, jt, :, 0:DK].bitcast(f32r),
                            psv_view.rearrange("p (h d) -> p h d", h=NH),
                        )
                return v_sb

            def emit_pv_stage(stg):
                b2, i2, hp2, jt, pvs, v_sb, pt, res_sb = stg
                for h in range(2):
                    nc.tensor.matmul(
                        pvs[h],
                        lhsT=v_sb[:, jt, 2 * hp2 + h, :].bitcast(f32r),
                        rhs=pt[:, h, :].bitcast(f32r),
                        start=(jt == 0),
                        stop=(jt == NJ - 1),
                    )
                if jt == NJ - 1:
                    on_group_complete(b2, i2, hp2, pvs, res_sb)

            def on_group_complete(b, i, hp, pvs, res_sb):
                # normalize: denominator replicated on rows 64..127
                for h in range(2):
                    rcp = smp.tile([DK, ISZ], f32, tag=f"rcp{h}", name=f"rcp{h}")
                    nc.vector.reciprocal(rcp, pvs[h][DK : 2 * DK, :])
                    nc.vector.tensor_mul(
                        res_sb[
                            h * DK : (h + 1) * DK, hp, i * ISZ : (i + 1) * ISZ
                        ].bitcast(f32r),
                        pvs[h][0:DK, :],
                        rcp,
                    )
                if hp == 1 and (b, i) not in _deferred_op:
                    if (b, i) in _op_tiles:  # kt=0 pre-emitted late-stream
                        tiles = _op_tiles[(b, i)]
                    else:
                        tiles = [None, None]
                        emit_outproj_kt(i, res_sb, 0, tiles)
                    emit_outproj_kt(i, res_sb, 1, tiles)
                    emit_outproj_tail(b, i, x_sbs[b], tiles)

            def emit_outproj_mt(b, i, mt):
                # full out-projection for one output row-tile: 2 accumulate
                # matmuls + bias/residual + store.  Used by the stage-hook
                # chunks that spread an i-boundary's out-projection.
                res_sb = res_sbs[b]
                x_sb = x_sbs[b]
                ps = psg.tile([P, ISZ], f32, tag="gen", name="gop")
                for kt in range(KT):
                    nc.tensor.matmul(
                        ps,
                        lhsT=wout_sb[:, kt, mt * P : (mt + 1) * P].bitcast(f32r),
                        rhs=res_sb[:, kt, i * ISZ : (i + 1) * ISZ].bitcast(f32r),
                        start=(kt == 0),
                        stop=(kt == KT - 1),
                    )
                y_sb = outp.tile([P, ISZ], f32, tag="y", name="y")
                nc.vector.scalar_tensor_tensor(
                    out=y_sb,
                    in0=ps,
                    scalar=beff_sb[:, mt : mt + 1],
                    in1=x_sb[:, mt, i * ISZ : (i + 1) * ISZ],
                    op0=ADD,
                    op1=ADD,
                )
                nc.sync.dma_start(
                    y_d[b, mt * P : (mt + 1) * P, i * ISZ : (i + 1) * ISZ],
                    y_sb,
                )

            def emit_outproj_kt(i, res_sb, kt, tiles):
                # one contraction step for both output row-tiles; kt=0 only
                # needs hp=0's normalized rows, so it can run while hp=1's
                # attention stream is still in flight
                for mt in range(KT):
                    if kt == 0:
                        tiles[mt] = psg.tile([P, ISZ], f32, tag="gen", name="gout")
                    nc.tensor.matmul(
                        tiles[mt],
                        lhsT=wout_sb[:, kt, mt * P : (mt + 1) * P].bitcast(f32r),
                        rhs=res_sb[:, kt, i * ISZ : (i + 1) * ISZ].bitcast(f32r),
                        start=(kt == 0),
                        stop=(kt == KT - 1),
                    )

            def emit_outproj_tail(b, i, x_sb, tiles):
                for mt in range(KT):
                    y_sb = outp.tile([P, ISZ], f32, tag="y", name="y")
                    nc.vector.scalar_tensor_tensor(
                        out=y_sb,
                        in0=tiles[mt],
                        scalar=beff_sb[:, mt : mt + 1],
                        in1=x_sb[:, mt, i * ISZ : (i + 1) * ISZ],
                        op0=ADD,
                        op1=ADD,
                    )
                    nc.sync.dma_start(
                        y_d[b, mt * P : (mt + 1) * P, i * ISZ : (i + 1) * ISZ],
                        y_sb,
                    )

            # globally software-pipelined attention: each stage emits
            # S^T + exp for (g, jt) and only then the PV of the previous
            # stage, so the in-order PE queue never stalls on an exp.
            # Group completions (normalize, out-proj, next batch's qk/v
            # generation) ride along when a stage's last PV is emitted.
            x_sbs = {0: emit_xload(0)}
            # minimal prologue: just q01/k01 on the first half of x and
            # the first v tiles, so exp starts as early as possible; all
            # remaining generation work is dispatched one chunk per
            # attention stage via _gen_sched below.
            qk_sbs = {0: emit_qkgen(0, x_sbs[0], i_list=[0], t_list=[0, 1])}
            v_sbs = {0: emit_vgen(0, x_sbs[0], jt_list=[0, 1, 2],
                                  ones_range=(0, 4))}
            res_sbs = {}
            pending = None
            # stage-hook schedule: (b, i, hp, jt) -> (target_batch, kind,
            # kwargs).  Batch 0's leftover generation spreads through the
            # pipeline-fill stages; batch b+1's full generation spreads one
            # chunk per steady-state stage of batch b's i=1 groups (the PE
            # has headroom there once warm; a lump would bubble ACT).
            _gen_sched = {
                (0, 0, 0, 0): (0, "v", dict(jt_list=[3])),
                (0, 0, 0, 1): (0, "qk", dict(i_list=[1], t_list=[1])),
                (0, 0, 0, 2): (0, "qk", dict(i_list=[0], t_list=[2])),
                (0, 0, 0, 3): (0, "qk", dict(i_list=[0], t_list=[3])),
                (0, 0, 0, 4): (0, "v", dict(jt_list=[4],
                                            ones_range=(4, 4))),
                (0, 0, 0, 5): (0, "v", dict(jt_list=[5])),
                (0, 0, 0, 6): (0, "v", dict(jt_list=[6])),
                (0, 0, 0, 7): (0, "v", dict(jt_list=[7])),
                (0, 0, 1, 1): (0, "qk", dict(i_list=[1], t_list=[3])),
                (0, 0, 1, 2): (0, "qk", dict(i_list=[1], t_list=[0])),
                (0, 0, 1, 3): (0, "qk", dict(i_list=[1], t_list=[2])),
            }
            # every i-boundary out-projection except the kernel-final one
            # is spread as two per-mt chunks into the stages right after its
            # group completes (the completion traces earlier in the same
            # stage, so the data dependency is already satisfied).  To free
            # those slots, four of batch b+1's qk chunks move into the empty
            # fill-phase slots of batch 0's second group.
            _deferred_op = set()
            _op_tiles = {}
            for nb in range(1, BPC):
                pb = nb - 1
                _deferred_op.add((pb, 0))
                _gen_sched[(pb, 1, 0, 0)] = (pb, "op", dict(i=0, mt=0))
                _gen_sched[(pb, 1, 0, 1)] = (pb, "op", dict(i=0, mt=1))
                _deferred_op.add((pb, NI - 1))
                _gen_sched[(nb, 0, 0, 0)] = (pb, "op", dict(i=NI - 1, mt=0))
                _gen_sched[(nb, 0, 0, 1)] = (pb, "op", dict(i=NI - 1, mt=1))
                if nb + 1 < BPC:
                    pass  # deeper pipelines would continue the pattern
            _deferred_op.add((BPC - 1, 0))
            _gen_sched[(BPC - 1, 1, 0, 0)] = (BPC - 1, "op", dict(i=0, mt=0))
            _gen_sched[(BPC - 1, 1, 0, 1)] = (BPC - 1, "op", dict(i=0, mt=1))
            _gen_sched[(BPC - 1, 1, 1, 6)] = (BPC - 1, "opk0", dict(i=1))
            for nb in range(1, BPC):
                pb = nb - 1
                # batch nb's qk chunks for i=0 ride the free fill-phase
                # slots (traced after x(nb) is loaded at batch pb's start)
                for tt in range(4):
                    _gen_sched[(0, 0, 1, 4 + tt)] = (
                        nb, "qk", dict(i_list=[0], t_list=[tt])
                    )
                chunks = [(nb, "v", dict(jt_list=[0], ones_range=(0, NJ)))]
                for tt in range(4):
                    chunks.append((nb, "qk", dict(i_list=[1], t_list=[tt])))
                for jj in range(1, NJ):
                    chunks.append((nb, "v", dict(jt_list=[jj])))
                # alternate qk and v chunks across the remaining free slots
                # of batch pb's i=1 groups to even the per-stage PE load
                order = []
                qks = [c for c in chunks if c[1] == "qk"]
                vs = [c for c in chunks if c[1] != "qk"]
                while qks or vs:
                    if qks:
                        order.append(qks.pop(0))
                    if vs:
                        order.append(vs.pop(0))
                slots = [
                    (pb, NI - 1, hp2, jj)
                    for hp2 in range(2)
                    for jj in range(NJ)
                    if (pb, NI - 1, hp2, jj) not in _gen_sched
                ]
                for slot, c in zip(slots, order):
                    _gen_sched[slot] = c
            for b in range(BPC):
                res_sbs[b] = resp.tile([P, KT, N], f32, tag="res", name=f"res{b}")
                for i in range(NI):
                    if i == 0 and b + 1 < BPC:
                        x_sbs[b + 1] = emit_xload(b + 1)
                    for hp in range(2):
                        q_t = qk_sbs[b][2 * hp]
                        k_t = qk_sbs[b][2 * hp + 1]
                        pvs = [
                            psv.tile([P, ISZ], f32, tag="pv", name=f"pv{h}")
                            for h in range(2)
                        ]
                        for jt in range(NJ):
                            st = pss.tile([P, 2, ISZ], f32, tag="st", name="st")
                            for h in range(2):
                                nc.tensor.matmul(
                                    st[:, h, :],
                                    lhsT=k_t[
                                        h * DK : (h + 1) * DK,
                                        jt * P : (jt + 1) * P,
                                    ].bitcast(f32r),
                                    rhs=q_t[
                                        h * DK : (h + 1) * DK,
                                        i * ISZ : (i + 1) * ISZ,
                                    ].bitcast(f32r),
                                )
                            pt = ptp.tile([P, 2, ISZ], f32, tag="pt", name="pt")
                            if (b, i, hp, jt) in dve_exp:
                                nc.vector.tensor_scalar(
                                    pt.bitcast(i32), st, SCH_A32, SCH_B32,
                                    MULT, ADD,
                                )
                            else:
                                nc.scalar.activation(pt.bitcast(f32r), st, EXP)
                            if pending is not None:
                                emit_pv_stage(pending)
                            pending = (
                                b, i, hp, jt, pvs, v_sbs[b], pt, res_sbs[b]
                            )
                            chunk = _gen_sched.pop((b, i, hp, jt), None)
                            if chunk is not None:
                                nb, kind, args = chunk
                                if kind == "op":
                                    emit_outproj_mt(nb, **args)
                                elif kind == "opk0":
                                    tiles = [None, None]
                                    _op_tiles[(nb, args["i"])] = tiles
                                    emit_outproj_kt(
                                        args["i"], res_sbs[nb], 0, tiles
                                    )
                                elif kind == "qk":
                                    if nb not in qk_sbs:
                                        qk_sbs[nb] = alloc_qk(nb)
                                    emit_qkgen(nb, x_sbs[nb],
                                               qk_sb=qk_sbs[nb], **args)
                                else:
                                    v_sbs[nb] = emit_vgen(
                                        nb, x_sbs[nb], **args
                                    )
            emit_pv_stage(pending)

    nc.compile()
    return nc


def _prep_weights(W_qkv, b_qkv, W_out, b_out):
    """Host-side weight reshuffles (cheap, [256, 768]-sized)."""
    Wr = np.ascontiguousarray(W_qkv, dtype=np.float32).reshape(C, NH, 3, DK)
    br = np.ascontiguousarray(b_qkv, dtype=np.float32).reshape(NH, 3, DK)
    # feature tiles: [q0|q1], [k0|k1], [q2|q3], [k2|k3]; q pre-scaled
    cols = []
    for hp in range(2):
        cols.append(Wr[:, 2 * hp, 0] * SCALE)
        cols.append(Wr[:, 2 * hp + 1, 0] * SCALE)
        cols.append(Wr[:, 2 * hp, 1])
        cols.append(Wr[:, 2 * hp + 1, 1])
    wqk = np.concatenate(cols, axis=1)  # [C, 512]
    bq = np.stack(
        [
            np.concatenate([br[2 * hp, 0], br[2 * hp + 1, 0]]) * SCALE
            for hp in range(2)
        ],
        axis=1,
    )  # [128, 2]
    wv = np.concatenate([Wr[:, h, 2] for h in range(NH)], axis=1)  # [C, 256]
    bv = np.concatenate([br[h, 2] for h in range(NH)])  # [256]
    W_out = np.ascontiguousarray(W_out, dtype=np.float32)
    b_eff = (b_out + bv @ W_out).astype(np.float32)  # [256]
    beff = b_eff.reshape(KT, P).T.copy()  # [128, 2] col=mt
    return (
        np.ascontiguousarray(wqk, dtype=np.float32),
        np.ascontiguousarray(bq, dtype=np.float32),
        np.ascontiguousarray(wv, dtype=np.float32),
        W_out,
        np.ascontiguousarray(beff, dtype=np.float32),
    )


def _device_inputs(x3, W_qkv, b_qkv, W_out, b_out):
    wqk, bq, wv, wout, beff = _prep_weights(
        np.asarray(W_qkv), np.asarray(b_qkv), np.asarray(W_out), np.asarray(b_out)
    )
    def to_pkm(w):  # [C, M] -> [P, KT, M] with row kt*P+p on (p, kt)
        return w.reshape(KT, P, -1).transpose(1, 0, 2)

    wall = np.ascontiguousarray(
        np.concatenate([to_pkm(wqk), to_pkm(wv), to_pkm(wout)], axis=2),
        dtype=np.float32,
    )
    sm = np.ascontiguousarray(
        np.concatenate([bq, beff, np.ones((P, DK), np.float32)], axis=1),
        dtype=np.float32,
    )
    in_maps = []
    for c in range(N_CORES):
        in_maps.append(
            {
                "x": np.ascontiguousarray(x3[c * BPC : (c + 1) * BPC]),
                "wall": wall,
                "sm": sm,
            }
        )
    return in_maps


DVE_EXP_STAGES = ()
KV_ACT = False


def kernel(x, W_qkv, b_qkv, W_out, b_out):
    from concourse.bass_utils import run_bass_kernel_spmd

    if "nc" not in _CACHE:
        _CACHE["nc"] = _build_module(dve_exp=DVE_EXP_STAGES, kv_act=KV_ACT)
    nc = _CACHE["nc"]

    x = np.ascontiguousarray(np.asarray(x), dtype=np.float32)
    Bx, Cx, Hx, Wx = x.shape
    x3 = x.reshape(Bx, Cx, Hx * Wx)
    in_maps = _device_inputs(x3, W_qkv, b_qkv, W_out, b_out)

    res = run_bass_kernel_spmd(nc, in_maps, core_ids=list(range(N_CORES)))
    y = np.concatenate([r["y"] for r in res.results], axis=0)  # [16, 256, 1024]
    return y.reshape(Bx, Cx, Hx, Wx).astype(np.float32)

